# revision 19
# baseline (speedup 1.0000x reference)
"""Trainium2 Bass kernel for the GCN graph classifier (2x GCNConv + mean-pool + linear).

Strategy (8 NeuronCores, SPMD):
- Nodes (and their incident in-edges) are sharded contiguously across the 8 cores;
  the small 128x128 weights are replicated.
- GCN layers are linear, so S @ (x @ W) is computed as (S @ x) @ W: propagate
  features first (per-edge gather + one-hot matmul scatter-add on the PE), then W.
- Features travel as fp8e4m3, prescaled by dinv[src] on the host (for x) or at the
  previous layer's drain (for r1), so the per-chunk selection matrix is a pure
  is_equal one-hot and each gather descriptor is a 128-byte payload read from
  256-byte-aligned pair rows (idx = row>>1, even/odd split by base offset).
- Both layers store node features in the same global "row-major by (core,
  partition, tile)" layout, so the two launches share identical index tables.
- Self-loops are excluded from the edge list; each tile adds its own (prescaled)
  feature block via a single identity-rhs matmul from a contiguous SBUF copy.
- Per 128-edge chunk: P[e, n] = (dst_local[e] == n) built on the vector engine,
  tensor engine accumulates aggT += M^T @ P into PSUM; dinv[dst] scaling, bias
  (rank-1 sqrt(deg) x b matmul), and relu fuse into one activation at drain.
- Two launches: layer 1 emits dinv-prescaled fp8 activations per shard; the host
  concatenates shards and feeds layer 2, which also does one-hot mean-pooling.
- Host side: index bookkeeping only (degrees, edge bucketing, int16 index packing,
  dtype/layout prep of inputs) plus the final 8-way partial reduction and the tiny
  [64,128] @ [128,2] classifier.
"""
import sys
from contextlib import ExitStack

import numpy as np
import ml_dtypes

for _p in ("/opt/trn_rl_repo", "/root/.axon_site/_ro/trn_rl_repo"):
    if _p not in sys.path:
        sys.path.append(_p)

import concourse.bass as bass
import concourse.bacc as bacc
import concourse.mybir as mybir
import concourse.tile as tile
from concourse import bass_utils
from concourse import ap_utils

F32 = mybir.dt.float32
BF16 = mybir.dt.bfloat16
FP8 = mybir.dt.float8e4
I16 = mybir.dt.int16

# ---- fixed problem geometry (50000 nodes, 800000 edges, 64 graphs, 128 feats)
NC = 8            # cores
NT = 49           # dst tiles of 128 nodes per core
CEV = 9           # chunks (128 edges) per tile with even feature-row src
COD = 9           # chunks per tile with odd feature-row src
NCH = CEV + COD
GRP = 7           # tiles per gather group (49 = 7*7)
NGRAPH = 64
F = 128
NPAD = NC * NT * 128          # 50176
NPC = NT * 128                # 6272 nodes per core
NPAIR = NPAD // 2             # 25088 pair rows of 256 fp8

# ramped gather groups: short first groups hide the pipeline-fill latency and
# short last groups shrink the post-final-gather compute tail
_GROUP_SIZES = (2, 2, 3, 4, 5, 6, 7, 7, 5, 3, 2, 2, 1)
assert sum(_GROUP_SIZES) == NT
_GROUPS = []
_t = 0
for _n in _GROUP_SIZES:
    _GROUPS.append((_t, _n))
    _t += _n


def _wrap16(arr_i16):
    """int16 [M*16] -> [128, M]: element i at [i%16, i//16], replicated across the
    8 GPSIMD Q7-core partition groups (HW reads its group's copy)."""
    total = arr_i16.shape[0]
    block = arr_i16.reshape(total // 16, 16).T
    return np.tile(block, (8, 1)).copy()


def _manual_gather(g, out_ap, in_ap, idxs_ap, num_idxs, elem_size, elem_step):
    """dma_gather with a sub-256B payload from 256B-strided rows (the bass wrapper
    only allows elem_size_bytes%256==0; the ISA constraint is on the row stride)."""
    assert idxs_ap.dtype == mybir.dt.int16
    assert in_ap.space == bass.MemorySpace.DRAM
    assert in_ap.ap[0][0] == elem_step
    stride_bytes = elem_step * mybir.dt.size(in_ap.dtype)
    stride_bytes_256 = stride_bytes // 256
    assert stride_bytes_256 * 256 == stride_bytes and stride_bytes_256 < 256
    assert ap_utils.ap_is_contiguous(out_ap.ap[1:])
    assert ap_utils.ap_is_contiguous(idxs_ap.ap[1:])
    _in_ap = g.lower_ap_dma(in_ap, for_custom_bir_dma=True)
    _idxs_ap = g.lower_ap(idxs_ap)
    _out_ap = g.lower_ap(out_ap)
    return g.add_instruction(
        mybir.InstDMAGatherAnt(
            name=g.bass.get_next_instruction_name(),
            ins=[*_in_ap, _idxs_ap, g.lower_val_access(g.to_reg(num_idxs))],
            outs=[_out_ap],
            transpose=False,
            num_idxs=num_idxs,
            elem_size=elem_size,
            stride_bytes_256=stride_bytes_256,
            gen_mode=0,
            single_packet=False,
            queue_num=0,
            sbuf_tokens_per_rank=0,
            sbuf_free_dim_per_rank=0,
            sbuf_free_dim_pad_per_rank=0,
            sbuf_byte_offset=0,
        )
    )


def _preprocess(x, edge_index, batch):
    N = x.shape[0]
    src = np.asarray(edge_index[0], dtype=np.int64)
    dst = np.asarray(edge_index[1], dtype=np.int64)

    # degree includes the self-loop the reference adds per node
    deg = (np.bincount(dst, minlength=NPAD) + 1).astype(np.float64)
    dinv = (1.0 / np.sqrt(deg)).astype(np.float32)
    sqd = np.sqrt(deg).astype(np.float32)

    # feature-row layout: node n=(c,t,p) stored at row(n) = c*6272 + p*49 + t
    n_all = np.arange(NPAD, dtype=np.int64)
    c_all, rem = n_all // NPC, n_all % NPC
    row_of = c_all * NPC + (rem % 128) * NT + rem // 128

    x_rm = np.zeros((NPAD, F), dtype=ml_dtypes.float8_e4m3)
    x_pre = np.asarray(x, dtype=np.float32) * dinv[:N, None]
    x_rm[row_of[:N]] = x_pre.astype(ml_dtypes.float8_e4m3)
    src8 = x_rm.reshape(NPAIR, 2 * F)

    srow = row_of[src]
    tile_of = dst >> 7
    order = np.argsort(tile_of, kind="stable")
    srow_s, dst_s, tile_s = srow[order], dst[order], tile_of[order]
    NTILES = NPAD // 128
    starts = np.searchsorted(tile_s, np.arange(NTILES))
    ends = np.searchsorted(tile_s, np.arange(NTILES), side="right")

    iota128 = np.broadcast_to(np.arange(128, dtype=np.float32), (128, 128)).astype(ml_dtypes.bfloat16)
    iota64 = np.broadcast_to(np.arange(NGRAPH, dtype=np.float32), (128, NGRAPH)).astype(ml_dtypes.bfloat16)
    ident = np.eye(128, dtype=np.float32).astype(ml_dtypes.bfloat16)

    batch_pad = np.full(NPAD, -1.0, dtype=np.float32)
    batch_pad[:N] = np.asarray(batch, dtype=np.float32)

    in_maps = []
    NPAT = 5          # selection patterns per tile-class: 4 shared pairs + leftover
    for c in range(NC):
        iev = np.zeros((NT, CEV * 128), dtype=np.int16)
        iod = np.zeros((NT, COD * 128), dtype=np.int16)
        lcol = np.full((NT, 2 * NPAT * 128), -1.0, dtype=np.float32)
        for t in range(NT):
            gt = c * NT + t
            s, e = starts[gt], ends[gt]
            er, ed = srow_s[s:e], dst_s[s:e]
            ev_m = (er & 1) == 0
            for cls in range(2):
                hr = er[ev_m] if cls == 0 else er[~ev_m]
                hd = ed[ev_m] if cls == 0 else ed[~ev_m]
                n = len(hr)
                assert n <= CEV * 128, f"overflow {n}"
                # sort by local dst; pair consecutive same-dst edges so chunk
                # pairs (2p, 2p+1) share one selection pattern
                ld = (hd - gt * 128).astype(np.int64)
                o = np.argsort(ld, kind="stable")
                hr, ld = (hr[o] >> 1).astype(np.int16), ld[o]
                k = np.bincount(ld, minlength=128) if n else np.zeros(128, np.int64)
                # mark pair members: within each dst run, positions 0..2*floor(k/2)
                pos = np.arange(n) - np.repeat(np.cumsum(k) - k, k) if n else np.zeros(0, np.int64)
                is_pair = pos < 2 * (np.repeat(k, k) // 2) if n else np.zeros(0, bool)
                # cap pairs at 512 doubles: excess goes to the leftover chunk
                pair_idx = np.nonzero(is_pair)[0]
                if len(pair_idx) > 1024:
                    is_pair[pair_idx[1024:]] = False
                e1 = np.nonzero(is_pair)[0][0::2]
                e2 = np.nonzero(is_pair)[0][1::2]
                rest = np.nonzero(~is_pair)[0]
                assert len(rest) <= 128, f"leftover overflow {len(rest)}"
                idx_arr = np.zeros(CEV * 128, dtype=np.int16)
                P = len(e1)
                for p in range((P + 127) // 128):
                    sl = slice(p * 128, min((p + 1) * 128, P))
                    m = sl.stop - sl.start
                    idx_arr[2 * p * 128:2 * p * 128 + m] = hr[e1[sl]]
                    idx_arr[(2 * p + 1) * 128:(2 * p + 1) * 128 + m] = hr[e2[sl]]
                    lcol[t, (cls * NPAT + p) * 128:(cls * NPAT + p) * 128 + m] = ld[e1[sl]]
                idx_arr[8 * 128:8 * 128 + len(rest)] = hr[rest]
                lcol[t, (cls * NPAT + 4) * 128:(cls * NPAT + 4) * 128 + len(rest)] = ld[rest]
                if cls == 0:
                    iev[t] = idx_arr
                else:
                    iod[t] = idx_arr
        nodes = np.arange(c * NPC, (c + 1) * NPC)
        in_maps.append({
            "iev": _wrap16(iev.reshape(-1)),
            "iod": _wrap16(iod.reshape(-1)),
            "lcol": lcol.reshape(NT * 2 * NPAT, 128).T.copy(),
            "ddst": dinv[nodes].reshape(NT, 128).T.copy(),
            "ddst2": (dinv[nodes] ** 2).reshape(NT, 128).T.copy(),
            "sqd": sqd[nodes].reshape(1, NPC).astype(ml_dtypes.bfloat16),
            "gcol": batch_pad[nodes].reshape(NT, 128).T.copy(),
            "selfx": np.ascontiguousarray(
                x_rm[c * NPC:(c + 1) * NPC].reshape(128, NT * F)),
            "io128": np.asarray(iota128), "io64": np.asarray(iota64),
            "ident": np.asarray(ident),
        })
    counts = np.bincount(np.asarray(batch, dtype=np.int64), minlength=NGRAPH).astype(np.float32)
    return src8, in_maps, counts


def _emit_layer(tc, outs, ins, li):
    """li=0: gather fp8 x_pre -> r1 (dinv-prescaled, fp8) shard out.
    li=1: gather fp8 r1 -> pool partials out."""
    nc = tc.nc
    Relu = mybir.ActivationFunctionType.Relu
    Copy = mybir.ActivationFunctionType.Copy
    ISEQ = mybir.AluOpType.is_equal

    ctx = ExitStack()
    const = ctx.enter_context(tc.tile_pool(name="const", bufs=1))
    gev = ctx.enter_context(tc.tile_pool(name="gev", bufs=3))
    god = ctx.enter_context(tc.tile_pool(name="god", bufs=3))
    ptp = ctx.enter_context(tc.tile_pool(name="ptp", bufs=20))
    small = ctx.enter_context(tc.tile_pool(name="small", bufs=8))
    work = ctx.enter_context(tc.tile_pool(name="work", bufs=4))
    psA = ctx.enter_context(tc.tile_pool(name="psA", bufs=4, space="PSUM"))
    psB = ctx.enter_context(tc.tile_pool(name="psB", bufs=2, space="PSUM"))
    psP = ctx.enter_context(tc.tile_pool(name="psP", bufs=1, space="PSUM"))

    names = ["iev", "iod", "lcol", "ddst", "sqd", "W", "b", "io128", "ident", "selfx"]
    if li == 1:
        names += ["gcol", "io64"]
    cs = {}
    for k in names:
        ap = ins[k]
        t = const.tile(list(ap.shape), ap.tensor.dtype, tag=k, name=f"c_{k}")
        if k in ("iev", "iod"):
            # split so the first gather groups' index slices land early
            cut = 7 * CEV * 8
            nc.sync.dma_start(t[:, :cut], ap[:, :cut])
            nc.sync.dma_start(t[:, cut:], ap[:, cut:])
        else:
            nc.sync.dma_start(t[:], ap[:])
        cs[k] = t

    src_ev, src_od = ins["src8"][:, 0:F], ins["src8"][:, F:2 * F]
    if li == 0:
        r1sb = const.tile([128, NT * F], FP8, tag="r1sb", name="r1sb")
    else:
        poolps = psP.tile([NGRAPH, F], F32, name="poolps")

    # Software-pipelined drains: a tile's PSUM drain + W/bias matmuls are emitted
    # DA tiles later (and the L2 pooling matmul DB tiles after that) so the PE's
    # depth-4 wait queue never blocks on the Act engine.
    DA, DB = 2, 1
    qA, qB = [], []

    def drainA(t, agg):
        aggs = work.tile([128, 128], BF16, tag="aggT", name="aggs")
        nc.scalar.activation(aggs[:], agg[:], Copy)
        outp = psB.tile([128, 128], F32, name="outp")
        nc.tensor.matmul(outp[:], lhsT=aggs[:], rhs=cs["W"][:], start=True, stop=False)
        nc.tensor.matmul(outp[:], lhsT=cs["sqd"][0:1, t * 128:(t + 1) * 128],
                         rhs=cs["b"][0:1, :], start=False, stop=True)
        if li == 0:
            # r1 = dinv*relu(dinv*aggW + b) = relu(dinv^2*aggW + dinv*b)
            nc.scalar.activation(r1sb[:, t * F:(t + 1) * F], outp[:], Relu,
                                 scale=cs["ddst"][:, t:t + 1])
            return None
        r2t = small.tile([128, F], BF16, tag="r2", name="r2t")
        nc.scalar.activation(r2t[:], outp[:], Relu, scale=cs["ddst"][:, t:t + 1])
        bt = small.tile([128, NGRAPH], BF16, tag="bt", name="bt")
        nc.vector.tensor_scalar(bt[:], cs["io64"][:], cs["gcol"][:, t:t + 1], None, ISEQ)
        return (t, r2t, bt)

    def drainB(t, r2t, bt):
        nc.tensor.matmul(poolps[:], lhsT=bt[:], rhs=r2t[:],
                         start=(t == 0), stop=(t == NT - 1))

    def stepA(t, agg):
        qA.append((t, agg))
        if len(qA) > DA:
            r = drainA(*qA.pop(0))
            if r is not None:
                qB.append(r)
                if len(qB) > DB:
                    drainB(*qB.pop(0))

    for (t0, ntg) in _GROUPS:
        nev, nod = ntg * CEV, ntg * COD
        ge = gev.tile([128, nev, F], FP8, tag="gev", name="ge")
        go = god.tile([128, nod, F], FP8, tag="god", name="go")
        _manual_gather(nc.gpsimd, ge[:], src_ev,
                       cs["iev"][:, t0 * CEV * 8:(t0 + ntg) * CEV * 8],
                       nev * 128, F, 2 * F)
        _manual_gather(nc.gpsimd, go[:], src_od,
                       cs["iod"][:, t0 * COD * 8:(t0 + ntg) * COD * 8],
                       nod * 128, F, 2 * F)

        for ti in range(ntg):
            t = t0 + ti
            agg = psA.tile([128, 128], F32, name="agg")
            first = True
            for cls in range(2):
                gsb = ge if cls == 0 else go
                base = ti * CEV
                for p in range(5):
                    q = t * 10 + cls * 5 + p
                    pt = ptp.tile([128, 128], BF16, tag="p", name="pt")
                    nc.vector.tensor_scalar(pt[:], cs["io128"][:], cs["lcol"][:, q:q + 1],
                                            None, ISEQ)
                    chunks = (2 * p, 2 * p + 1) if p < 4 else (8,)
                    for ck in chunks:
                        nc.tensor.matmul(agg[:], lhsT=gsb[:, base + ck, :], rhs=pt[:],
                                         start=first, stop=False)
                        first = False
            # self-loop last: aggT[:, d] += x_pre[node(c,t,d)] via identity rhs
            nc.tensor.matmul(agg[:], lhsT=cs["selfx"][:, t * F:(t + 1) * F],
                             rhs=cs["ident"][:], start=False, stop=True)
            stepA(t, agg)

    while qA:
        r = drainA(*qA.pop(0))
        if r is not None:
            qB.append(r)
    while qB:
        drainB(*qB.pop(0))

    if li == 0:
        cuts = [0, 12 * F, 24 * F, 36 * F, NT * F]
        for a, b in zip(cuts, cuts[1:]):
            nc.sync.dma_start(outs["r1c"][:, a:b], r1sb[:, a:b])
    else:
        pool_sb = work.tile([NGRAPH, F], F32, tag="pool", name="pool_sb")
        nc.vector.tensor_copy(pool_sb[:], poolps[:])
        nc.sync.dma_start(outs["pool"][:, :], pool_sb[:])
    ctx.close()


_BUILT = {}


def _build(li):
    if li in _BUILT:
        return _BUILT[li]
    nc = bacc.Bacc("TRN2", target_bir_lowering=False, debug=False, num_devices=NC)
    specs = {
        "src8": ([NPAIR, 2 * F], FP8),
        "selfx": ([128, NT * F], FP8),
        "iev": ([128, NT * CEV * 8], I16),
        "iod": ([128, NT * COD * 8], I16),
        "lcol": ([128, NT * 10], F32),
        "ddst": ([128, NT], F32),
        "sqd": ([1, NPC], BF16),
        "W": ([F, F], BF16), "b": ([1, F], BF16),
        "io128": ([128, 128], BF16),
        "ident": ([128, 128], BF16),
    }
    if li == 1:
        specs["gcol"] = ([128, NT], F32)
        specs["io64"] = ([128, NGRAPH], BF16)
    ins = {k: nc.dram_tensor(k, shp, dt, kind="ExternalInput").ap()
           for k, (shp, dt) in specs.items()}
    if li == 0:
        outs = {"r1c": nc.dram_tensor("r1c", [128, NT * F], FP8, kind="ExternalOutput").ap()}
    else:
        outs = {"pool": nc.dram_tensor("pool", [NGRAPH, F], F32, kind="ExternalOutput").ap()}
    with tile.TileContext(nc) as tc:
        _emit_layer(tc, outs, ins, li)
    nc.compile()
    _BUILT[li] = nc
    return nc


def kernel(x, edge_index, batch, W1, b1, W2, b2, Wc, bc, _trace=False):
    x = np.asarray(x)
    src8, in_maps, counts = _preprocess(x, edge_index, batch)

    m1 = []
    for m in in_maps:
        m1.append({k: m[k] for k in ["iev", "iod", "lcol", "sqd", "selfx", "io128", "ident"]}
                  | {"src8": src8,
                     "ddst": m["ddst2"],
                     "W": np.asarray(W1, np.float32).astype(ml_dtypes.bfloat16),
                     "b": np.asarray(b1, np.float32).astype(ml_dtypes.bfloat16).reshape(1, F)})
    nc1 = _build(0)
    import time as _time
    _t0 = _time.time()
    res1 = bass_utils.run_bass_kernel_spmd(nc1, m1, core_ids=list(range(NC)), trace=_trace)
    _t1 = _time.time()
    r1c = [np.asarray(res1.results[c]["r1c"]) for c in range(NC)]
    # rows are (c, p, t)-ordered: stacking the per-core [128, NT*128] blocks and
    # reshaping yields the global feature-row matrix
    r1_rm = np.stack(r1c).reshape(NPAD, F)
    r18 = r1_rm.reshape(NPAIR, 2 * F)

    m2 = []
    for c, m in enumerate(in_maps):
        m2.append({k: m[k] for k in ["iev", "iod", "lcol", "ddst", "sqd", "gcol", "io128", "io64", "ident"]}
                  | {"src8": r18,
                     "selfx": r1c[c],
                     "W": np.asarray(W2, np.float32).astype(ml_dtypes.bfloat16),
                     "b": np.asarray(b2, np.float32).astype(ml_dtypes.bfloat16).reshape(1, F)})
    nc2 = _build(1)
    _t2 = _time.time()
    res2 = bass_utils.run_bass_kernel_spmd(nc2, m2, core_ids=list(range(NC)), trace=_trace)
    _t3 = _time.time()
    kernel._launch_walls = (_t1 - _t0, _t3 - _t2)

    if _trace:
        kernel._last = (res1, res2)
    pooled = np.sum(np.stack([np.asarray(res2.results[c]["pool"], np.float64)
                              for c in range(NC)]), axis=0)
    pooled /= np.maximum(counts, 1.0)[:, None]
    out = pooled @ np.asarray(Wc, np.float64) + np.asarray(bc, np.float64)
    return out.astype(np.float32)


# revision 20
# speedup vs baseline: 1.0051x; 1.0051x over previous
"""Trainium2 Bass kernel for the GCN graph classifier (2x GCNConv + mean-pool + linear).

Strategy (8 NeuronCores, SPMD):
- Nodes (and their incident in-edges) are sharded contiguously across the 8 cores;
  the small 128x128 weights are replicated.
- GCN layers are linear, so S @ (x @ W) is computed as (S @ x) @ W: propagate
  features first (per-edge gather + one-hot matmul scatter-add on the PE), then W.
- Features travel as fp8e4m3, prescaled by dinv[src] on the host (for x) or at the
  previous layer's drain (for r1), so the per-chunk selection matrix is a pure
  is_equal one-hot and each gather descriptor is a 128-byte payload read from
  256-byte-aligned pair rows (idx = row>>1, even/odd split by base offset).
- Both layers store node features in the same global "row-major by (core,
  partition, tile)" layout, so the two launches share identical index tables.
- Self-loops are excluded from the edge list; each tile adds its own (prescaled)
  feature block via a single identity-rhs matmul from a contiguous SBUF copy.
- Per 128-edge chunk: P[e, n] = (dst_local[e] == n) built on the vector engine,
  tensor engine accumulates aggT += M^T @ P into PSUM; dinv[dst] scaling, bias
  (rank-1 sqrt(deg) x b matmul), and relu fuse into one activation at drain.
- Two launches: layer 1 emits dinv-prescaled fp8 activations per shard; the host
  concatenates shards and feeds layer 2, which also does one-hot mean-pooling.
- Host side: index bookkeeping only (degrees, edge bucketing, int16 index packing,
  dtype/layout prep of inputs) plus the final 8-way partial reduction and the tiny
  [64,128] @ [128,2] classifier.
"""
import sys
from contextlib import ExitStack

import numpy as np
import ml_dtypes

for _p in ("/opt/trn_rl_repo", "/root/.axon_site/_ro/trn_rl_repo"):
    if _p not in sys.path:
        sys.path.append(_p)

import concourse.bass as bass
import concourse.bacc as bacc
import concourse.mybir as mybir
import concourse.tile as tile
from concourse import bass_utils
from concourse import ap_utils

F32 = mybir.dt.float32
BF16 = mybir.dt.bfloat16
FP8 = mybir.dt.float8e4
I16 = mybir.dt.int16

# ---- fixed problem geometry (50000 nodes, 800000 edges, 64 graphs, 128 feats)
NC = 8            # cores
NT = 49           # dst tiles of 128 nodes per core
CEV = 9           # chunks (128 edges) per tile with even feature-row src
COD = 9           # chunks per tile with odd feature-row src
NCH = CEV + COD
GRP = 7           # tiles per gather group (49 = 7*7)
NGRAPH = 64
F = 128
NPAD = NC * NT * 128          # 50176
NPC = NT * 128                # 6272 nodes per core
NPAIR = NPAD // 2             # 25088 pair rows of 256 fp8

# ramped gather groups: short first groups hide the pipeline-fill latency and
# short last groups shrink the post-final-gather compute tail
_GROUP_SIZES = (2, 2, 3, 4, 5, 6, 7, 7, 5, 3, 2, 2, 1)
assert sum(_GROUP_SIZES) == NT
_GROUPS = []
_t = 0
for _n in _GROUP_SIZES:
    _GROUPS.append((_t, _n))
    _t += _n


def _wrap16(arr_i16):
    """int16 [M*16] -> [128, M]: element i at [i%16, i//16], replicated across the
    8 GPSIMD Q7-core partition groups (HW reads its group's copy)."""
    total = arr_i16.shape[0]
    block = arr_i16.reshape(total // 16, 16).T
    return np.tile(block, (8, 1)).copy()


def _manual_gather(g, out_ap, in_ap, idxs_ap, num_idxs, elem_size, elem_step):
    """dma_gather with a sub-256B payload from 256B-strided rows (the bass wrapper
    only allows elem_size_bytes%256==0; the ISA constraint is on the row stride)."""
    assert idxs_ap.dtype == mybir.dt.int16
    assert in_ap.space == bass.MemorySpace.DRAM
    assert in_ap.ap[0][0] == elem_step
    stride_bytes = elem_step * mybir.dt.size(in_ap.dtype)
    stride_bytes_256 = stride_bytes // 256
    assert stride_bytes_256 * 256 == stride_bytes and stride_bytes_256 < 256
    assert ap_utils.ap_is_contiguous(out_ap.ap[1:])
    assert ap_utils.ap_is_contiguous(idxs_ap.ap[1:])
    _in_ap = g.lower_ap_dma(in_ap, for_custom_bir_dma=True)
    _idxs_ap = g.lower_ap(idxs_ap)
    _out_ap = g.lower_ap(out_ap)
    return g.add_instruction(
        mybir.InstDMAGatherAnt(
            name=g.bass.get_next_instruction_name(),
            ins=[*_in_ap, _idxs_ap, g.lower_val_access(g.to_reg(num_idxs))],
            outs=[_out_ap],
            transpose=False,
            num_idxs=num_idxs,
            elem_size=elem_size,
            stride_bytes_256=stride_bytes_256,
            gen_mode=0,
            single_packet=False,
            queue_num=0,
            sbuf_tokens_per_rank=0,
            sbuf_free_dim_per_rank=0,
            sbuf_free_dim_pad_per_rank=0,
            sbuf_byte_offset=0,
        )
    )


def _preprocess(x, edge_index, batch):
    N = x.shape[0]
    src = np.asarray(edge_index[0], dtype=np.int64)
    dst = np.asarray(edge_index[1], dtype=np.int64)

    # degree includes the self-loop the reference adds per node
    deg = (np.bincount(dst, minlength=NPAD) + 1).astype(np.float64)
    dinv = (1.0 / np.sqrt(deg)).astype(np.float32)
    sqd = np.sqrt(deg).astype(np.float32)

    # feature-row layout: node n=(c,t,p) stored at row(n) = c*6272 + p*49 + t
    n_all = np.arange(NPAD, dtype=np.int64)
    c_all, rem = n_all // NPC, n_all % NPC
    row_of = c_all * NPC + (rem % 128) * NT + rem // 128

    x_rm = np.zeros((NPAD, F), dtype=ml_dtypes.float8_e4m3)
    x_pre = np.asarray(x, dtype=np.float32) * dinv[:N, None]
    x_rm[row_of[:N]] = x_pre.astype(ml_dtypes.float8_e4m3)
    src8 = x_rm.reshape(NPAIR, 2 * F)

    srow = row_of[src]
    tile_of = dst >> 7
    order = np.argsort(tile_of, kind="stable")
    srow_s, dst_s, tile_s = srow[order], dst[order], tile_of[order]
    NTILES = NPAD // 128
    starts = np.searchsorted(tile_s, np.arange(NTILES))
    ends = np.searchsorted(tile_s, np.arange(NTILES), side="right")

    iota128 = np.broadcast_to(np.arange(128, dtype=np.float32), (128, 128)).astype(ml_dtypes.bfloat16)
    iota64 = np.broadcast_to(np.arange(NGRAPH, dtype=np.float32), (128, NGRAPH)).astype(ml_dtypes.bfloat16)
    ident = np.eye(128, dtype=np.float32).astype(ml_dtypes.bfloat16)

    batch_pad = np.full(NPAD, -1.0, dtype=np.float32)
    batch_pad[:N] = np.asarray(batch, dtype=np.float32)

    in_maps = []
    NPAT = 5          # selection patterns per tile-class: 4 shared pairs + leftover
    for c in range(NC):
        iev = np.zeros((NT, CEV * 128), dtype=np.int16)
        iod = np.zeros((NT, COD * 128), dtype=np.int16)
        lcol = np.full((NT, 2 * NPAT * 128), -1.0, dtype=np.float32)
        for t in range(NT):
            gt = c * NT + t
            s, e = starts[gt], ends[gt]
            er, ed = srow_s[s:e], dst_s[s:e]
            ev_m = (er & 1) == 0
            for cls in range(2):
                hr = er[ev_m] if cls == 0 else er[~ev_m]
                hd = ed[ev_m] if cls == 0 else ed[~ev_m]
                n = len(hr)
                assert n <= CEV * 128, f"overflow {n}"
                # sort by local dst; pair consecutive same-dst edges so chunk
                # pairs (2p, 2p+1) share one selection pattern
                ld = (hd - gt * 128).astype(np.int64)
                o = np.argsort(ld, kind="stable")
                hr, ld = (hr[o] >> 1).astype(np.int16), ld[o]
                k = np.bincount(ld, minlength=128) if n else np.zeros(128, np.int64)
                # mark pair members: within each dst run, positions 0..2*floor(k/2)
                pos = np.arange(n) - np.repeat(np.cumsum(k) - k, k) if n else np.zeros(0, np.int64)
                is_pair = pos < 2 * (np.repeat(k, k) // 2) if n else np.zeros(0, bool)
                # cap pairs at 512 doubles: excess goes to the leftover chunk
                pair_idx = np.nonzero(is_pair)[0]
                if len(pair_idx) > 1024:
                    is_pair[pair_idx[1024:]] = False
                e1 = np.nonzero(is_pair)[0][0::2]
                e2 = np.nonzero(is_pair)[0][1::2]
                rest = np.nonzero(~is_pair)[0]
                assert len(rest) <= 128, f"leftover overflow {len(rest)}"
                idx_arr = np.zeros(CEV * 128, dtype=np.int16)
                P = len(e1)
                for p in range((P + 127) // 128):
                    sl = slice(p * 128, min((p + 1) * 128, P))
                    m = sl.stop - sl.start
                    idx_arr[2 * p * 128:2 * p * 128 + m] = hr[e1[sl]]
                    idx_arr[(2 * p + 1) * 128:(2 * p + 1) * 128 + m] = hr[e2[sl]]
                    lcol[t, (cls * NPAT + p) * 128:(cls * NPAT + p) * 128 + m] = ld[e1[sl]]
                idx_arr[8 * 128:8 * 128 + len(rest)] = hr[rest]
                lcol[t, (cls * NPAT + 4) * 128:(cls * NPAT + 4) * 128 + len(rest)] = ld[rest]
                if cls == 0:
                    iev[t] = idx_arr
                else:
                    iod[t] = idx_arr
        nodes = np.arange(c * NPC, (c + 1) * NPC)
        in_maps.append({
            "iev": _wrap16(iev.reshape(-1)),
            "iod": _wrap16(iod.reshape(-1)),
            "lcol": lcol.reshape(NT * 2 * NPAT, 128).T.copy(),
            "ddst": dinv[nodes].reshape(NT, 128).T.copy(),
            "ddst2": (dinv[nodes] ** 2).reshape(NT, 128).T.copy(),
            "sqd": sqd[nodes].reshape(1, NPC).astype(ml_dtypes.bfloat16),
            "gcol": batch_pad[nodes].reshape(NT, 128).T.copy(),
            "selfx": np.ascontiguousarray(
                x_rm[c * NPC:(c + 1) * NPC].reshape(128, NT * F)),
            "io128": np.asarray(iota128), "io64": np.asarray(iota64),
            "ident": np.asarray(ident),
        })
    counts = np.bincount(np.asarray(batch, dtype=np.int64), minlength=NGRAPH).astype(np.float32)
    return src8, in_maps, counts


def _emit_layer(tc, outs, ins, li):
    """li=0: gather fp8 x_pre -> r1 (dinv-prescaled, fp8) shard out.
    li=1: gather fp8 r1 -> pool partials out."""
    nc = tc.nc
    Relu = mybir.ActivationFunctionType.Relu
    Copy = mybir.ActivationFunctionType.Copy
    ISEQ = mybir.AluOpType.is_equal

    ctx = ExitStack()
    const = ctx.enter_context(tc.tile_pool(name="const", bufs=1))
    gev = ctx.enter_context(tc.tile_pool(name="gev", bufs=4))
    god = ctx.enter_context(tc.tile_pool(name="god", bufs=4))
    ptp = ctx.enter_context(tc.tile_pool(name="ptp", bufs=20))
    small = ctx.enter_context(tc.tile_pool(name="small", bufs=8))
    work = ctx.enter_context(tc.tile_pool(name="work", bufs=4))
    psA = ctx.enter_context(tc.tile_pool(name="psA", bufs=5, space="PSUM"))
    psB = ctx.enter_context(tc.tile_pool(name="psB", bufs=2, space="PSUM"))
    psP = ctx.enter_context(tc.tile_pool(name="psP", bufs=1, space="PSUM"))

    names = ["iev", "iod", "lcol", "ddst", "sqd", "W", "b", "io128", "ident", "selfx"]
    if li == 1:
        names += ["gcol", "io64"]
    cs = {}
    for k in names:
        ap = ins[k]
        t = const.tile(list(ap.shape), ap.tensor.dtype, tag=k, name=f"c_{k}")
        if k in ("iev", "iod"):
            # split so the first gather groups' index slices land early
            cut = 7 * CEV * 8
            nc.sync.dma_start(t[:, :cut], ap[:, :cut])
            nc.sync.dma_start(t[:, cut:], ap[:, cut:])
        else:
            nc.sync.dma_start(t[:], ap[:])
        cs[k] = t

    src_ev, src_od = ins["src8"][:, 0:F], ins["src8"][:, F:2 * F]
    if li == 0:
        r1sb = const.tile([128, NT * F], FP8, tag="r1sb", name="r1sb")
    else:
        poolps = psP.tile([NGRAPH, F], F32, name="poolps")

    # Software-pipelined drains: a tile's PSUM drain + W/bias matmuls are emitted
    # DA tiles later (and the L2 pooling matmul DB tiles after that) so the PE's
    # depth-4 wait queue never blocks on the Act engine.
    DA, DB = 3, 1
    qA, qB = [], []

    def drainA(t, agg):
        aggs = work.tile([128, 128], BF16, tag="aggT", name="aggs")
        nc.scalar.activation(aggs[:], agg[:], Copy)
        outp = psB.tile([128, 128], F32, name="outp")
        nc.tensor.matmul(outp[:], lhsT=aggs[:], rhs=cs["W"][:], start=True, stop=False)
        nc.tensor.matmul(outp[:], lhsT=cs["sqd"][0:1, t * 128:(t + 1) * 128],
                         rhs=cs["b"][0:1, :], start=False, stop=True)
        if li == 0:
            # r1 = dinv*relu(dinv*aggW + b) = relu(dinv^2*aggW + dinv*b)
            nc.scalar.activation(r1sb[:, t * F:(t + 1) * F], outp[:], Relu,
                                 scale=cs["ddst"][:, t:t + 1])
            return None
        r2t = small.tile([128, F], BF16, tag="r2", name="r2t")
        nc.scalar.activation(r2t[:], outp[:], Relu, scale=cs["ddst"][:, t:t + 1])
        bt = small.tile([128, NGRAPH], BF16, tag="bt", name="bt")
        nc.vector.tensor_scalar(bt[:], cs["io64"][:], cs["gcol"][:, t:t + 1], None, ISEQ)
        return (t, r2t, bt)

    def drainB(t, r2t, bt):
        nc.tensor.matmul(poolps[:], lhsT=bt[:], rhs=r2t[:],
                         start=(t == 0), stop=(t == NT - 1))

    def stepA(t, agg):
        qA.append((t, agg))
        if len(qA) > DA:
            r = drainA(*qA.pop(0))
            if r is not None:
                qB.append(r)
                if len(qB) > DB:
                    drainB(*qB.pop(0))

    for (t0, ntg) in _GROUPS:
        nev, nod = ntg * CEV, ntg * COD
        ge = gev.tile([128, nev, F], FP8, tag="gev", name="ge")
        go = god.tile([128, nod, F], FP8, tag="god", name="go")
        _manual_gather(nc.gpsimd, ge[:], src_ev,
                       cs["iev"][:, t0 * CEV * 8:(t0 + ntg) * CEV * 8],
                       nev * 128, F, 2 * F)
        _manual_gather(nc.gpsimd, go[:], src_od,
                       cs["iod"][:, t0 * COD * 8:(t0 + ntg) * COD * 8],
                       nod * 128, F, 2 * F)

        for ti in range(ntg):
            t = t0 + ti
            agg = psA.tile([128, 128], F32, name="agg")
            first = True
            for cls in range(2):
                gsb = ge if cls == 0 else go
                base = ti * CEV
                for p in range(5):
                    q = t * 10 + cls * 5 + p
                    pt = ptp.tile([128, 128], BF16, tag="p", name="pt")
                    nc.vector.tensor_scalar(pt[:], cs["io128"][:], cs["lcol"][:, q:q + 1],
                                            None, ISEQ)
                    chunks = (2 * p, 2 * p + 1) if p < 4 else (8,)
                    for ck in chunks:
                        nc.tensor.matmul(agg[:], lhsT=gsb[:, base + ck, :], rhs=pt[:],
                                         start=first, stop=False)
                        first = False
            # self-loop last: aggT[:, d] += x_pre[node(c,t,d)] via identity rhs
            nc.tensor.matmul(agg[:], lhsT=cs["selfx"][:, t * F:(t + 1) * F],
                             rhs=cs["ident"][:], start=False, stop=True)
            stepA(t, agg)

    while qA:
        r = drainA(*qA.pop(0))
        if r is not None:
            qB.append(r)
    while qB:
        drainB(*qB.pop(0))

    if li == 0:
        cuts = [0, 12 * F, 24 * F, 36 * F, NT * F]
        for a, b in zip(cuts, cuts[1:]):
            nc.sync.dma_start(outs["r1c"][:, a:b], r1sb[:, a:b])
    else:
        pool_sb = work.tile([NGRAPH, F], F32, tag="pool", name="pool_sb")
        nc.vector.tensor_copy(pool_sb[:], poolps[:])
        nc.sync.dma_start(outs["pool"][:, :], pool_sb[:])
    ctx.close()


_BUILT = {}


def _build(li):
    if li in _BUILT:
        return _BUILT[li]
    nc = bacc.Bacc("TRN2", target_bir_lowering=False, debug=False, num_devices=NC)
    specs = {
        "src8": ([NPAIR, 2 * F], FP8),
        "selfx": ([128, NT * F], FP8),
        "iev": ([128, NT * CEV * 8], I16),
        "iod": ([128, NT * COD * 8], I16),
        "lcol": ([128, NT * 10], F32),
        "ddst": ([128, NT], F32),
        "sqd": ([1, NPC], BF16),
        "W": ([F, F], BF16), "b": ([1, F], BF16),
        "io128": ([128, 128], BF16),
        "ident": ([128, 128], BF16),
    }
    if li == 1:
        specs["gcol"] = ([128, NT], F32)
        specs["io64"] = ([128, NGRAPH], BF16)
    ins = {k: nc.dram_tensor(k, shp, dt, kind="ExternalInput").ap()
           for k, (shp, dt) in specs.items()}
    if li == 0:
        outs = {"r1c": nc.dram_tensor("r1c", [128, NT * F], FP8, kind="ExternalOutput").ap()}
    else:
        outs = {"pool": nc.dram_tensor("pool", [NGRAPH, F], F32, kind="ExternalOutput").ap()}
    with tile.TileContext(nc) as tc:
        _emit_layer(tc, outs, ins, li)
    nc.compile()
    _BUILT[li] = nc
    return nc


def kernel(x, edge_index, batch, W1, b1, W2, b2, Wc, bc, _trace=False):
    x = np.asarray(x)
    src8, in_maps, counts = _preprocess(x, edge_index, batch)

    m1 = []
    for m in in_maps:
        m1.append({k: m[k] for k in ["iev", "iod", "lcol", "sqd", "selfx", "io128", "ident"]}
                  | {"src8": src8,
                     "ddst": m["ddst2"],
                     "W": np.asarray(W1, np.float32).astype(ml_dtypes.bfloat16),
                     "b": np.asarray(b1, np.float32).astype(ml_dtypes.bfloat16).reshape(1, F)})
    nc1 = _build(0)
    import time as _time
    _t0 = _time.time()
    res1 = bass_utils.run_bass_kernel_spmd(nc1, m1, core_ids=list(range(NC)), trace=_trace)
    _t1 = _time.time()
    r1c = [np.asarray(res1.results[c]["r1c"]) for c in range(NC)]
    # rows are (c, p, t)-ordered: stacking the per-core [128, NT*128] blocks and
    # reshaping yields the global feature-row matrix
    r1_rm = np.stack(r1c).reshape(NPAD, F)
    r18 = r1_rm.reshape(NPAIR, 2 * F)

    m2 = []
    for c, m in enumerate(in_maps):
        m2.append({k: m[k] for k in ["iev", "iod", "lcol", "ddst", "sqd", "gcol", "io128", "io64", "ident"]}
                  | {"src8": r18,
                     "selfx": r1c[c],
                     "W": np.asarray(W2, np.float32).astype(ml_dtypes.bfloat16),
                     "b": np.asarray(b2, np.float32).astype(ml_dtypes.bfloat16).reshape(1, F)})
    nc2 = _build(1)
    _t2 = _time.time()
    res2 = bass_utils.run_bass_kernel_spmd(nc2, m2, core_ids=list(range(NC)), trace=_trace)
    _t3 = _time.time()
    kernel._launch_walls = (_t1 - _t0, _t3 - _t2)

    if _trace:
        kernel._last = (res1, res2)
    pooled = np.sum(np.stack([np.asarray(res2.results[c]["pool"], np.float64)
                              for c in range(NC)]), axis=0)
    pooled /= np.maximum(counts, 1.0)[:, None]
    out = pooled @ np.asarray(Wc, np.float64) + np.asarray(bc, np.float64)
    return out.astype(np.float32)
